# revision 10
# baseline (speedup 1.0000x reference)
"""GNN message-passing layer on 8 TRN2 NeuronCores.

Reference computation (N=16384, D=128):
    a    = adj_mat.astype(f32)            # [N, N]
    deg  = a.sum(axis=0)                  # [N]
    agg  = (a^T @ x) / deg[:, None]       # [N, D]
    out  = relu(agg @ U^T)[None]          # [1, N, D]

Sharding: column-shard adj_mat across the 8 cores (core c owns output
nodes i in [c*2048, (c+1)*2048) and reads adj[:, islice]); x and U are
replicated. The contraction over j (all 16384 rows) is then fully local
to each core — no collective is needed, and each core emits its own
contiguous slice of the output.

Host-side staging (part of the sharding step, all value-lossless):
  - adj shard -> fp8e4 [16384, 2048]  ({0,1} exact; quarters HBM
    traffic vs int32)
  - x -> fp16 in [p, jb, d] layout so the device DMA is one contiguous
    transfer; U -> U^T fp16.

Per-core kernel (v3):
  - adj shard streamed in 16 chunks of 8 row-blocks [128, 8, 2048] fp8.
  - Per chunk, first a deg burst: per row-block, four col-group-packed
    matmuls (ones [128,32] stationary at tile_position (0,32*ic))
    accumulate deg into one PSUM bank as [128, 512], each 32-partition
    group holding one 512-i chunk's deg replicated 32x. The four run
    concurrently on the PE column groups (~512 cols of streaming per
    row-block instead of 2048). Then the agg matmuls (full-array,
    N=512, 4 PSUM banks). Batching pays the full-array<->col-tiled PE
    transition twice per chunk instead of twice per row-block, and the
    deg burst needs no x, so the PE starts as soon as adjacency lands.
  - A warmup train of matmuls on memset fp8 data runs into a scratch
    PSUM bank while the DMA queues initialize, so the HAM clock gate
    is already 8/8 (2.4 GHz) when real data arrives.
  - During the last chunk's agg stream: reciprocal of the deg bank
    (DVE, [128,512] in one op — the x32 replication makes the layout
    free), then per 512-chunk an exact broadcast matmul (stationary
    filled with 1/32 so 32 identical replicas sum back to the value)
    replicates 1/deg to all 128 partitions, drained to SBUF.
  - finale per 512-chunk: h = matmul(lhsT=U^T, rhs=aggT chunk) -> psum
    [e, i], then ONE fused DVE op relu+scale:
    out = (h max 0) mult rdeg_bc  (scalar_tensor_tensor), and the
    256 KB chunk DMAs out immediately. Output leaves as [e, i] which
    the host transposes.

fp8 is exact for the adjacency and deg; x/U fp16 rounding gives ~3e-4
relative error. All accumulation is fp32 in PSUM.
"""

import sys

if "/opt/trn_rl_repo" not in sys.path:
    sys.path.insert(0, "/opt/trn_rl_repo")

import numpy as np

from concourse import bacc, mybir, tile
from concourse.bass import ts
from concourse.bass_utils import run_bass_kernel_spmd

N = 16384  # nodes
D = 128  # features
CORES = 8
S = N // CORES  # 2048 output nodes per core
P = 128  # partitions
JB = N // P  # 128 row-blocks
IC = S // 512  # 4 moving-dim chunks of 512

F16 = mybir.dt.float16
F32 = mybir.dt.float32
F8 = mybir.dt.float8e4


def build_nc():
    nc = bacc.Bacc("TRN2", target_bir_lowering=False, debug=False)

    a_dram = nc.dram_tensor("a", [N, S], F8, kind="ExternalInput").ap()
    x_dram = nc.dram_tensor("x", [P, JB * D], F16, kind="ExternalInput").ap()
    ut_dram = nc.dram_tensor("ut", [D, D], F16, kind="ExternalInput").ap()
    # [e, i] layout; host transposes to [2048, 128]
    out_dram = nc.dram_tensor("out", [D, S], F32, kind="ExternalOutput").ap()

    with tile.TileContext(nc) as tc:
        CH = 16  # row-blocks per adj DMA chunk (4 MB fp8 transfers)
        NCK = JB // CH
        with (
            tc.tile_pool(name="persist", bufs=1) as persist,
            tc.tile_pool(name="adj", bufs=3) as adj_pool,
        ):
            xh = persist.tile([P, JB, D], F16)
            ut16 = persist.tile([D, D], F16)
            # fp8 ones stationary for the M=32 deg matmuls
            ones32 = persist.tile([P, 32], F8)
            nc.gpsimd.memset(ones32[:], 1.0)
            # 1/32 stationary: broadcasts one 32-replica group to 128
            # partitions exactly (32 * v/32 sums bit-exact in fp32)
            avg32 = persist.tile([P, P], F8)
            nc.gpsimd.memset(avg32[:], 1.0 / 32.0)
            # scratch moving operand for the PE warmup train
            warm = persist.tile([P, 512], F8)
            nc.gpsimd.memset(warm[:], 1.0)

            ag16 = persist.tile([P, S], F16)
            # fp16 so the fp8 broadcast matmul accepts it as rhs (fp32
            # pairing is rejected); 1/deg ~ 1.2e-4 is fp16-normal
            rdeg_all = persist.tile([P, 512], F16)
            rdeg_bc = persist.tile([P, IC, 512], F32)
            o_chunks = [
                persist.tile([P, 512], F32, name=f"o_chunk{i}") for i in range(IC)
            ]

            with tc.tile_pool(name="wps", bufs=1, space="PSUM") as wps:
                ps_warm = wps.tile([P, 512], F32, name="ps_warm")
                # ~4us of throwaway matmuls: keeps the PE busy through
                # the DMA-queue init (and the adj pipeline fill) so HAM
                # reaches 8/8 before real work
                for _ in range(20):
                    nc.tensor.matmul(
                        ps_warm[0:32, :], ones32[:], warm[:], start=True, stop=True
                    )

            with (
                tc.tile_pool(name="mmps", bufs=1, space="PSUM") as mmps,
                tc.tile_pool(name="bcps", bufs=1, space="PSUM") as bcps,
                tc.tile_pool(name="fps", bufs=2, space="PSUM") as fps,
            ):
                ps_agg = [mmps.tile([P, 512], F32, name=f"ps_agg{i}") for i in range(IC)]
                ps_deg = mmps.tile([P, 512], F32, name="ps_deg")

                def finale(ic):
                    h_ps = fps.tile([P, 512], F32, tag="h")
                    nc.tensor.matmul(
                        h_ps[:], ut16[:], ag16[:, ts(ic, 512)], start=True, stop=True
                    )
                    # out = relu(h) * (1/deg) in one DVE op (relu commutes
                    # with the positive per-column scale)
                    nc.vector.scalar_tensor_tensor(
                        o_chunks[ic][:],
                        h_ps[:],
                        0.0,
                        rdeg_bc[:, ic, :],
                        mybir.AluOpType.max,
                        mybir.AluOpType.mult,
                    )
                    eng = nc.sync if ic % 2 == 0 else nc.scalar
                    eng.dma_start(out_dram[:, ts(ic, 512)], o_chunks[ic][:])

                x_r = x_dram.rearrange("p (jb d) -> p jb d", jb=JB)
                for ck in range(NCK):
                    af = adj_pool.tile([P, CH, S], F8, tag="af")
                    # alternate the two HWDGE rings; keep ring 1 (scalar)
                    # busy with the x/ut prologue during the first chunks
                    eng = nc.sync if ck % 2 == 0 else nc.scalar
                    nc_src = a_dram[ck * CH * P : (ck + 1) * CH * P, :]
                    src_r = nc_src.rearrange("(c p) i -> p c i", p=P)
                    if ck == 0:
                        # split the first chunk so the opening matmuls are
                        # not gated on a full 4 MB transfer
                        eng.dma_start(af[:, 0:2, :], src_r[:, 0:2, :])
                        eng.dma_start(af[:, 2:, :], src_r[:, 2:, :])
                    else:
                        eng.dma_start(af[:], src_r)
                    # x prologue on ring 1: slice g gates the agg matmuls
                    # from row-block 32*g; the first 8 row-blocks land
                    # separately so chunk 0's agg isn't gated on 1 MB
                    if ck == 0:
                        nc.scalar.dma_start(xh[:, 0:8, :], x_r[:, 0:8, :])
                        nc.scalar.dma_start(ut16[:], ut_dram[:])
                        nc.scalar.dma_start(xh[:, 8:32, :], x_r[:, 8:32, :])
                    elif ck in (1, 3, 5):
                        g = (ck + 1) // 2
                        nc.scalar.dma_start(
                            xh[:, ts(g, 32), :], x_r[:, ts(g, 32), :]
                        )
                    # deg burst first: depends only on the adj chunk
                    for c in range(CH):
                        jb = ck * CH + c
                        first, last = jb == 0, jb == JB - 1
                        for ic in range(IC):
                            nc.tensor.matmul(
                                ps_deg[32 * ic : 32 * ic + 32, :],
                                ones32[:],
                                af[:, c, ts(ic, 512)],
                                start=first,
                                stop=last,
                                tile_position=(0, 32 * ic),
                            )
                    if ck < NCK - 1:
                        for c in range(CH):
                            jb = ck * CH + c
                            for ic in range(IC):
                                nc.tensor.matmul(
                                    ps_agg[ic][:],
                                    xh[:, jb, :],
                                    af[:, c, ts(ic, 512)],
                                    start=jb == 0,
                                    stop=False,
                                )
                    else:
                        # deg is complete; the reciprocal + broadcast chain
                        # hides under the last chunk's agg stream
                        with nc.allow_low_precision(
                            reason="1/deg ~ 1.2e-4 is fp16-normal; 5e-4 rel err ok"
                        ):
                            nc.vector.reciprocal(rdeg_all[:], ps_deg[:])
                        for ic in range(IC):
                            bc = bcps.tile([P, 512], F32, tag="bc")
                            nc.tensor.matmul(
                                bc[:],
                                avg32[32 * ic : 32 * ic + 32, :],
                                rdeg_all[32 * ic : 32 * ic + 32, :],
                                start=True,
                                stop=True,
                                # auto-derive caps at 64; the 4th row
                                # strip must be explicit
                                tile_position=(32 * ic, 0),
                            )
                            nc.vector.tensor_copy(rdeg_bc[:, ic, :], bc[:])
                        # last chunk runs ic-major so each PSUM bank's
                        # accumulation closes early: its drain, U-matmul,
                        # fused relu-scale and output DMA all overlap the
                        # remaining agg stream instead of tailing the run
                        for ic in range(IC):
                            for c in range(CH):
                                jb = ck * CH + c
                                nc.tensor.matmul(
                                    ps_agg[ic][:],
                                    xh[:, jb, :],
                                    af[:, c, ts(ic, 512)],
                                    start=False,
                                    stop=c == CH - 1,
                                )
                            # drain on ScalarE (DVE owns the finale ops)
                            nc.scalar.copy(ag16[:, ts(ic, 512)], ps_agg[ic][:])
                            if ic >= 1:
                                finale(ic - 1)
                        finale(IC - 1)

    nc.compile()
    return nc


_NC = None


def _get_nc():
    global _NC
    if _NC is None:
        _NC = build_nc()
    return _NC


def prep_in_maps(x, adj_mat, U):
    import ml_dtypes

    x = np.asarray(x, dtype=np.float32)
    adj_mat = np.asarray(adj_mat)
    U = np.asarray(U, dtype=np.float32)
    # x -> fp16 [p, jb, d] flattened to [128, JB*D]
    xt = np.ascontiguousarray(
        x.reshape(JB, P, D).transpose(1, 0, 2).astype(np.float16).reshape(P, JB * D)
    )
    ut = np.ascontiguousarray(U.T.astype(np.float16))
    # adjacency values are {0,1}: exact in fp8e4m3, and the int8 bit
    # patterns 0x00/0x38 can be produced by a table lookup (much faster
    # than a float astype over 1 GiB)
    lut = np.zeros(2, dtype=np.uint8)
    lut[1] = np.array(1.0, dtype=ml_dtypes.float8_e4m3).view(np.uint8)
    in_maps = []
    for c in range(CORES):
        a8 = lut[adj_mat[:, c * S : (c + 1) * S]].view(ml_dtypes.float8_e4m3)
        in_maps.append({"a": a8, "x": xt, "ut": ut})
    return in_maps


def assemble_out(results):
    # per-core out is [128, 2048] in [e, i] layout
    parts = []
    for c in range(CORES):
        parts.append(np.ascontiguousarray(results[c]["out"].T))
    return np.concatenate(parts, axis=0)[None]


def kernel(x, adj_mat, U, **_):
    nc = _get_nc()
    in_maps = prep_in_maps(x, adj_mat, U)
    res = run_bass_kernel_spmd(nc, in_maps, core_ids=list(range(CORES)))
    return assemble_out(res.results)
